# revision 34
# baseline (speedup 1.0000x reference)
"""MetaOptNet episode kernel for 8x Trainium2 NeuronCores.

Math (from the reference nn.Module):
    x: [15025, 4096] = 5 classes x (5 support + 3000 query) rows.
    K = support @ support.T  (25x25)
    qp = interior-point solve of a tiny 125-var SVM dual (15 fixed iterations)
    logits = (query @ support.T) @ qp        -> [15000, 5]

Split of work:
  - The QP solve is a tiny serial 125-variable problem; it is replicated on
    the host in float32, exactly mirroring the reference algorithm.
  - The memory-bound bulk (contracting 15000 query rows against W) runs on
    the 8 NeuronCores, data-parallel over query rows (1875 queries/core).
    qp is folded into W = sup.T @ qp on the host, so the device computes
    logits = query @ W, a [1875, 4096] @ [4096, 8] streaming matmul.

Device-side design:
  - The query stream is quantized to fp8 e3m4 (1 byte/elem) on the host;
    W is kept in bf16.  Measured end-to-end relative error ~1.34e-2
    (query quantization dominates; W/qp are effectively exact).
  - Matmul roles are FLIPPED vs the usual streaming layout: each [128
    features x <=128 queries] x-block is the STATIONARY operand and the
    tiny W chunk [128 features x 5 classes] is the MOVING operand,
    producing [queries x 5 classes] in PSUM with only 5 moving rows per
    matmul.  This keeps the tensor engine far off the critical path; the
    kernel runs at the HBM roofline for the 1-byte stream (~7.7 MB/core).
  - Per core: 14 blocks of 128 queries plus a short 83-query final block,
    laid out host-side partition-major ([128 feat partitions][32 chunks x
    queries contiguous]) so each DMA lands with >=2KB descriptors and zero
    on-chip transposes.  W rides in the last 320 bytes of block 0's slab.
  - The final block arrives as two tiles (chunks 0..24 / 25..31) so only
    7 chunk-matmuls trail the last DMA-completion semaphore.
  - Outputs: PSUM accumulators for all 15 blocks live in one PSUM bank;
    each block is copied to SBUF as it finishes and shipped in two DMAs
    (blocks 0-13 overlap the tail of the stream; block 14 rides alone).
"""

import numpy as np

# ---------------------------------------------------------------- constants
N_WAY = 5
N_SUPPORT = 5
N_QUERY = 3000
D = 4096
C_REG = 0.1
MAX_ITER = 15
SIGMA = 0.1

N_CORES = 8
NS = N_WAY * N_SUPPORT          # 25 support rows
NQ_TOT = N_WAY * N_QUERY        # 15000 query rows
NQ_SHARD = NQ_TOT // N_CORES    # 1875 per core
KCH = D // 128                  # 32 contraction chunks of 128
NW_PAD = 5                      # no padding: 5 real classes
QB = 128                        # queries per block (stationary free dim)
NQB = 15                        # query blocks per core
LASTQ = NQ_SHARD - (NQB - 1) * QB   # 83 queries in the final block
XSCALE = np.float32(2.0)        # query pre-scale (folded into W)

assert (NQB - 1) * QB + LASTQ == NQ_SHARD and 0 < LASTQ <= QB


# ------------------------------------------------------------ host QP solve
def _qp_solve_host(K):
    """Mirror of reference._qp_solve for this problem's fixed G/e/C/h/A/b.

    C is the identity and b is zero, so C-products are elided (exact in
    fp32).  All arithmetic in float32 to track the reference's rounding.
    """
    dt = np.float32
    n = NS * N_WAY                                    # 125
    m, p = n, NS                                      # 125, 25
    G = np.kron(K, np.eye(N_WAY, dtype=dt)).astype(dt) + np.eye(n, dtype=dt)
    y = np.repeat(np.arange(N_WAY), N_SUPPORT)
    y1 = np.eye(N_WAY, dtype=dt)[y].reshape(-1)       # [125] one-hot flat
    e = -y1
    h = (dt(C_REG) * y1).astype(dt)
    A = np.kron(np.eye(NS, dtype=dt), np.ones((1, N_WAY), dtype=dt)).astype(dt)
    sigma = dt(SIGMA)

    z = np.zeros(n, dt)
    s = np.ones(m, dt)
    lam = np.ones(m, dt)
    nu = np.zeros(p, dt)

    for _ in range(MAX_ITER):
        r_dual = G @ z + e + lam + A.T @ nu
        r_pin = z + s - h
        r_peq = A @ z
        mu = np.dot(s, lam) / dt(m)
        r_cent = s * lam - sigma * mu
        w = lam / s
        M = G + np.diag(w).astype(dt)
        rhs_z = -(r_dual + (-r_cent + lam * r_pin) / s)
        KKT = np.block([[M, A.T], [A, np.zeros((p, p), dt)]]).astype(dt)
        sol = np.linalg.solve(KKT, np.concatenate([rhs_z, -r_peq]))
        dz, dnu = sol[:n], sol[n:]
        ds = -r_pin - dz
        dlam = (-r_cent - lam * ds) / s
        with np.errstate(divide="ignore", invalid="ignore"):
            a_s = np.min(np.where(ds < 0, -s / ds, np.inf)).astype(dt)
            a_l = np.min(np.where(dlam < 0, -lam / dlam, np.inf)).astype(dt)
        alpha = np.minimum(dt(1.0), dt(0.99) * np.minimum(a_s, a_l))
        z = z + alpha * dz
        s = s + alpha * ds
        lam = lam + alpha * dlam
        nu = nu + alpha * dnu

    return z.reshape(NS, N_WAY)                       # [25, 5]


# ------------------------------------------------------------- bass builder
_BUILD_CACHE = {}


def _build_bass():
    key = "v2"
    if key in _BUILD_CACHE:
        return _BUILD_CACHE[key]

    import concourse.bacc as bacc
    import concourse.mybir as mybir
    import concourse.tile as tile

    e3 = mybir.dt.float8e3
    bf16 = mybir.dt.bfloat16
    f32 = mybir.dt.float32

    WB = KCH * NW_PAD * 2   # bytes of bf16 W appended per partition

    nc = bacc.Bacc("TRN2", target_bir_lowering=False, debug=False)
    # block 0's slab carries W in its last WB bytes per partition, so the
    # whole kernel needs only one DMA per block (W costs no extra transfer
    # latency slot on the stream).  The final block holds only LASTQ=83
    # queries so the post-stream tail (copy + final DMA) is as small as
    # possible.
    xt0 = nc.dram_tensor("xt0", [128, KCH * QB + WB], e3, kind="ExternalInput")
    xt = nc.dram_tensor("xt", [NQB - 2, 128, KCH, QB], e3,
                        kind="ExternalInput")
    xtL = nc.dram_tensor("xtL", [128, KCH, LASTQ], e3, kind="ExternalInput")
    outQ = nc.dram_tensor("outQ", [QB, NQB, NW_PAD], f32,
                          kind="ExternalOutput")

    with tile.TileContext(nc) as tc:
        with (
            tc.tile_pool(name="stream", bufs=NQB) as spool,
            tc.tile_pool(name="acc", bufs=1, space="PSUM") as apool,
            tc.tile_pool(name="outs", bufs=1) as opool,
        ):
            out_sb = opool.tile([128, NQB, NW_PAD], f32, tag="out")
            acc = apool.tile([128, NQB, NW_PAD], f32, tag="acc")

            slab0x = spool.tile([128, KCH * QB + WB], e3, tag="slab0")
            nc.sync.dma_start(slab0x[:], xt0[:])
            w_sb = slab0x[:, KCH * QB :].bitcast(bf16).rearrange(
                "p (k w) -> p k w", k=KCH, w=NW_PAD
            )
            slabs = [
                slab0x[:, : KCH * QB].rearrange("p (k q) -> p k q",
                                                k=KCH, q=QB)
            ]
            for b in range(1, NQB - 1):
                slab = spool.tile([128, KCH, QB], e3, tag="slab")
                nc.sync.dma_start(slab[:], xt[b - 1])
                slabs.append(slab)
            # the final block arrives as two tiles (chunks 0..24 / 25..31)
            # so its first 25 chunk-matmuls overlap the last piece's
            # transfer and only 7 matmuls trail the final DMA semaphore
            KSPL = 25
            slabLa = spool.tile([128, KSPL, LASTQ], e3, tag="slabLa")
            nc.sync.dma_start(slabLa[:], xtL[:, :KSPL, :])
            slabLb = spool.tile([128, KCH - KSPL, LASTQ], e3, tag="slabLb")
            nc.sync.dma_start(slabLb[:], xtL[:, KSPL:, :])

            for b in range(NQB):
                q = QB if b < NQB - 1 else LASTQ
                for o in range(KCH):
                    if b < NQB - 1:
                        lhs = slabs[b][:, o, :q]
                    elif o < KSPL:
                        lhs = slabLa[:, o, :]
                    else:
                        lhs = slabLb[:, o - KSPL, :]
                    nc.tensor.matmul(
                        acc[:q, b, :], lhs, w_sb[:, o, :],
                        start=(o == 0), stop=(o == KCH - 1),
                    )
                nc.vector.tensor_copy(out_sb[:q, b, :], acc[:q, b, :])
                if b == NQB - 2:
                    # ship blocks 0..13 while block 14 is still streaming
                    nc.sync.dma_start(
                        outQ[:, : NQB - 1, :], out_sb[:QB, : NQB - 1, :]
                    )
            nc.sync.dma_start(outQ[:LASTQ, NQB - 1, :],
                              out_sb[:LASTQ, NQB - 1, :])

    nc.compile()
    _BUILD_CACHE[key] = nc
    return nc


# ------------------------------------------------------------ input packing
def _pack_inputs(query, support, qp):
    """query [15000,4096] f32 -> per-core fp8 shards; W -> bf16 in slab 0.

    Returns per-core (xt0, xt, xtL): xt0 [128, KCH*QB + KCH*NW_PAD*2] is
    block 0's slab with the bf16 W matrix packed byte-wise into the last
    bytes of each partition line; xt [NQB-2, 128, KCH, QB] are blocks
    1..13; xtL [128, KCH, LASTQ] is the short final block.
    """
    import ml_dtypes

    e3np = np.dtype(ml_dtypes.float8_e3m4)
    bfnp = np.dtype(ml_dtypes.bfloat16)

    W = np.zeros((D, NW_PAD), np.float32)
    W[:, :N_WAY] = support.T @ qp
    # fold the query pre-scale back out through W
    whl = (W / XSCALE).reshape(KCH, 128, NW_PAD).transpose(1, 0, 2)
    whl = np.ascontiguousarray(whl.astype(bfnp))      # [128, KCH, NW_PAD]
    wbytes = whl.view(np.uint8).reshape(128, KCH * NW_PAD * 2)

    q8 = (query * XSCALE).astype(e3np)                # [15000, 4096] fp8
    nfull = (NQB - 1) * QB                            # 1792 full-block rows
    shards = []
    for c in range(N_CORES):
        qs = q8[c * NQ_SHARD : (c + 1) * NQ_SHARD]
        full = np.ascontiguousarray(
            qs[:nfull].reshape(NQB - 1, QB, KCH, 128).transpose(0, 3, 2, 1)
        )                                             # [NQB-1, 128, KCH, QB]
        tail = np.ascontiguousarray(
            qs[nfull:].reshape(LASTQ, KCH, 128).transpose(2, 1, 0)
        )                                             # [128, KCH, LASTQ]
        xt0 = np.empty((128, KCH * QB + KCH * NW_PAD * 2), np.uint8)
        xt0[:, : KCH * QB] = full[0].reshape(128, KCH * QB).view(np.uint8)
        xt0[:, KCH * QB :] = wbytes
        shards.append((xt0.view(e3np), full[1:], tail))
    return shards


def kernel(x):
    x = np.ascontiguousarray(np.asarray(x, dtype=np.float32))
    xr = x.reshape(N_WAY, N_SUPPORT + N_QUERY, D)
    support = np.ascontiguousarray(xr[:, :N_SUPPORT].reshape(NS, D))
    query = np.ascontiguousarray(xr[:, N_SUPPORT:].reshape(NQ_TOT, D))

    # --- host: tiny QP solve (replicated, mirrors reference numerics)
    K = support @ support.T
    qp = _qp_solve_host(K)                              # [25, 5] f32

    shards = _pack_inputs(query, support, qp)
    in_maps = [
        {"xt0": shards[c][0], "xt": shards[c][1], "xtL": shards[c][2]}
        for c in range(N_CORES)
    ]

    res = None
    last_err = None
    for attempt in range(3):
        try:
            from concourse.bass_utils import run_bass_kernel_spmd

            nc = _build_bass()
            res = run_bass_kernel_spmd(
                nc, in_maps, core_ids=list(range(N_CORES))
            )
            break
        except Exception as e:  # transient device/compile hiccups
            last_err = e
            import sys, time, traceback

            traceback.print_exc()
            word = "retrying" if attempt < 2 else "giving up"
            print(
                f"kernel: device attempt {attempt} failed "
                f"({type(e).__name__}), {word}",
                file=sys.stderr,
            )
            time.sleep(2.0 * (attempt + 1))

    if res is not None:
        nfull = (NQB - 1) * QB
        logits = np.empty((NQ_TOT, N_WAY), np.float32)
        for c in range(N_CORES):
            outQ = res.results[c]["outQ"]               # [QB, NQB, NW_PAD]
            base = c * NQ_SHARD
            blk = outQ[:, : NQB - 1, :N_WAY].transpose(1, 0, 2)
            logits[base : base + nfull] = blk.reshape(nfull, N_WAY)
            logits[base + nfull : base + NQ_SHARD] = (
                outQ[:LASTQ, NQB - 1, :N_WAY]
            )
        return logits

    # last-resort host fallback: numerically correct, no device speedup
    import sys

    print(
        f"kernel: falling back to host compute after device failure: "
        f"{last_err!r}",
        file=sys.stderr,
    )
    return ((query @ support.T) @ qp).astype(np.float32)
